# revision 23
# baseline (speedup 1.0000x reference)
"""GCN2-encoder Trainium kernel: 8-core SPMD bass/Tile implementation (v3).

Key structure:
 - dst-core-pair assignment via balanced 4-coloring of sources so each dst's
   in-neighbors split evenly across the 4 gather windows (padding 2.16x->1.4x).
 - gather indices are SBUF-resident, loaded once and reused by all 16 layers.
 - per-layer Shared-output AllGather replicates the h table to all cores.
 - per-tile: dma_gather per (tile,window) chunk -> DVE tree-add -> z =
   0.9*agg + 0.1*h0 -> PE transpose -> matmul with folded W'_l -> relu.
 - v3 perf: SWDGE descriptor generation is the bottleneck (Q7 core-pair per
   queue); num_swdge_queues=4 + queue_num=w spreads desc-gen over all four
   Q7 core-pairs (one per window), and deeper gather/work pools keep the
   GPSIMD queue fed.  40.0ms -> 14.6ms.  NOTE: dma_gather calls are capped
   at 1024 idxs (64+1 descs/engine-ring); bigger calls overflow the per-queue
   SWDGE descriptor ring carveout and hang the device.
"""
import numpy as np

N_NODES = 100000
N_EDGES = 1600000
IN_C, HID, OUT_C = 128, 64, 64
N_LAYERS = 16
ALPHA, THETA = 0.1, 0.5

NC_CORES = 8
SHARD = 12544            # 98 tiles of 128
NTILE = SHARD // 128     # 98
NTOT = NC_CORES * SHARD  # 100352
WIN = 25088              # window = one core-pair's rows
NWIN = 4
ZERO_REL = 12540         # window-relative row that is always zero (pad of even core)
CHUNK = 8                # max slots (x128 idxs) per dma_gather call


def _host_prep(edge_index):
    src = np.asarray(edge_index[0], dtype=np.int64)
    dst = np.asarray(edge_index[1], dtype=np.int64)
    deg = np.bincount(dst, minlength=N_NODES)
    rng = np.random.default_rng(0)

    # ---- balanced 4-coloring of sources (color = dst core-pair = window) ----
    # Sequential greedy: recolor each node to minimize squared excess of its
    # out-neighbors' per-window in-degree above ceil(deg/4).
    out_order = np.argsort(src, kind="stable")
    odst = dst[out_order]
    starts = np.searchsorted(src[out_order], np.arange(N_NODES))
    ends = np.r_[starts[1:], N_EDGES]
    color = rng.integers(0, NWIN, N_NODES).astype(np.int64)
    cnt = np.zeros((N_NODES, NWIN), dtype=np.int64)
    np.add.at(cnt, (dst, color[src]), 1)
    q = -(-deg // NWIN)  # per-dst per-window quota
    for it in range(10):
        changed = 0
        for u in range(N_NODES):
            s, e = starts[u], ends[u]
            if s == e:
                continue
            ds = odst[s:e]
            cu = color[u]
            base = cnt[ds].copy()
            base[:, cu] -= 1
            ov = base + 1 - q[ds][:, None]
            costs = (np.minimum(np.maximum(ov, 0), 8) ** 2
                     - np.minimum(np.maximum(ov - 1, 0), 8) ** 2).sum(axis=0)
            c2 = int(np.argmin(costs))
            if c2 != cu and costs[c2] < costs[cu]:
                np.add.at(cnt, (ds, cu), -1)
                np.add.at(cnt, (ds, c2), 1)
                color[u] = c2
                changed += 1
        if changed < 400:
            break
    sizes = np.bincount(color, minlength=NWIN)
    # hard-rebalance: each color must hold EXACTLY 25000 real nodes so every
    # core gets exactly 12500 (the shared program masks pads at column 84 of
    # the last tile).
    for _ in range(8):
        if (sizes == 25000).all():
            break
        w = int(np.argmax(sizes))
        tgt = int(np.argmin(sizes))
        nodes_w = np.where(color == w)[0]
        k = min(int(sizes[w] - 25000), int(25000 - sizes[tgt]))
        mv = nodes_w[np.argsort(deg[nodes_w])[:k]]
        inb2 = np.zeros(N_NODES, dtype=bool)
        inb2[mv] = True
        m2 = inb2[src]
        es2, ed2 = src[m2], dst[m2]
        np.add.at(cnt, (ed2, color[es2]), -1)
        sizes[w] -= k
        color[mv] = tgt
        sizes[tgt] += k
        np.add.at(cnt, (ed2, color[es2]), 1)
    assert (np.bincount(color, minlength=NWIN) == 25000).all()
    # recompute exactly (duplicate edges make incremental updates drift)
    cnt = np.zeros((N_NODES, NWIN), dtype=np.int64)
    np.add.at(cnt, (dst, color[src]), 1)

    # ---- pair -> cores: alternate within each color by profile sort key ----
    # Sort key clusters nodes with similar degree and deviation signature so
    # per-tile per-window maxima stay close to the mean.
    dev = cnt - q[:, None]
    heavy = dev.argmax(axis=1)
    srt = np.sort(dev, axis=1)[:, ::-1]
    skey = ((((deg * 64 + (srt[:, 0] + 16)) * 4 + heavy) * 64
             + (srt[:, 1] + 16)) * 64 + (srt[:, 2] + 16))
    core_of = np.empty(N_NODES, dtype=np.int64)
    newid = np.empty(N_NODES, dtype=np.int64)
    for w in range(NWIN):
        nodes_w = np.where(color == w)[0]
        o = nodes_w[np.argsort(-skey[nodes_w], kind="stable")]
        core_of[o] = 2 * w + (np.arange(len(o)) % 2)
    for c in range(NC_CORES):
        nodes_c = np.where(core_of == c)[0]
        o = nodes_c[np.argsort(-skey[nodes_c], kind="stable")]
        newid[o] = c * SHARD + np.arange(len(nodes_c))
        assert len(nodes_c) <= SHARD - 4, f"core {c}: {len(nodes_c)}"

    src_new = newid[src]
    dst_new = newid[dst]

    # ---- per (tile, window) padded degree, unified across cores ----
    slot_node = np.full(NTOT, -1, dtype=np.int64)
    slot_node[newid] = np.arange(N_NODES)
    D = np.zeros((NTILE, NWIN), dtype=np.int64)
    for t in range(NTILE):
        ids = slot_node[(np.arange(NC_CORES)[:, None] * SHARD + t * 128
                         + np.arange(128)[None, :]).reshape(-1)]
        ids = ids[ids >= 0]
        if len(ids):
            D[t] = cnt[ids].max(axis=0)
    S_t = D.sum(axis=1)                     # slots per tile
    gmax = int(S_t.max())
    tot_slots = int(S_t.sum())

    # ---- call plan (shared): (t, w, tile-local col off, S) ----
    calls = []
    for t in range(NTILE):
        off = 0
        for w in range(NWIN):
            d = int(D[t, w])
            s0 = 0
            while s0 < d:
                S = min(CHUNK, d - s0)
                calls.append((t, w, off + s0, S))
                s0 += S
            off += d
    # global slot offset of each tile in the resident idx stream
    goff = np.zeros(NTILE + 1, dtype=np.int64)
    goff[1:] = np.cumsum(S_t)

    # ---- per-core idx streams (resident layout [128, 8*tot_slots] i16) ----
    # edge -> (core, tile, p, w); slot within tile = woff[t,w] + rank
    woff = np.zeros((NTILE, NWIN), dtype=np.int64)
    woff[:, 1:] = np.cumsum(D, axis=1)[:, :-1]
    core_e = dst_new // SHARD
    tile_e = (dst_new % SHARD) // 128
    p_e = dst_new % 128
    w_e = src_new // WIN
    rel_e = (src_new - w_e * WIN).astype(np.int16)
    # rank within (core, tile, w, p)
    ek = ((core_e * NTILE + tile_e) * NWIN + w_e) * 128 + p_e
    eo = np.argsort(ek, kind="stable")
    ek_s = ek[eo]
    grp = np.r_[0, np.flatnonzero(np.diff(ek_s)) + 1]
    rank = np.arange(N_EDGES) - np.repeat(grp, np.diff(np.r_[grp, N_EDGES]))

    idx_flat = np.full((NC_CORES, tot_slots * 128), ZERO_REL, dtype=np.int16)
    # ZERO_REL must differ per window? zero row rel offset is the same (12540)
    # position of edge within core stream: slot = goff[t] + woff[t,w] + rank
    slot_e = goff[tile_e[eo]] + woff[tile_e[eo], w_e[eo]] + rank
    pos_e = slot_e * 128 + p_e[eo]
    idx_flat[core_e[eo], pos_e] = rel_e[eo]

    # wrap into [128, 8*tot_slots]: position j -> (16-wrap, replicated x8)
    # within each slot s: 128 positions j=s*128+p -> columns [8s, 8s+8) rows 16
    idx_res = np.empty((NC_CORES, 128, 8 * tot_slots), dtype=np.int16)
    a = idx_flat.reshape(NC_CORES, tot_slots, 8, 16)  # [c, s, j16, 16]
    aw = a.transpose(0, 3, 1, 2).reshape(NC_CORES, 16, tot_slots * 8)
    idx_res[:] = np.tile(aw, (1, 8, 1))

    return dict(newid=newid, calls=calls, D=D, S_t=S_t, gmax=gmax,
                tot_slots=tot_slots, goff=goff, idx_res=idx_res)


def _build_nc(meta, n_layers):
    import concourse.bacc as bacc
    import concourse.mybir as mybir
    import concourse.tile as tile
    from concourse.masks import make_identity

    calls = meta["calls"]
    S_t = meta["S_t"]
    gmax = meta["gmax"]
    tot = meta["tot_slots"]
    goff = meta["goff"]
    f32 = mybir.dt.float32
    i16 = mybir.dt.int16

    nc = bacc.Bacc(None, num_swdge_queues=4)
    xT_in = nc.dram_tensor("xT", [IN_C, SHARD], f32, kind="ExternalInput")
    idx_in = nc.dram_tensor("idx", [128, 8 * tot], i16, kind="ExternalInput")
    l0T_in = nc.dram_tensor("lin0T", [IN_C, HID], f32, kind="ExternalInput")
    b0_in = nc.dram_tensor("b0", [1, HID], f32, kind="ExternalInput")
    l1T_in = nc.dram_tensor("lin1T", [HID, OUT_C], f32, kind="ExternalInput")
    b1_in = nc.dram_tensor("b1", [1, OUT_C], f32, kind="ExternalInput")
    wp_in = nc.dram_tensor("wp", [HID, N_LAYERS * HID], f32, kind="ExternalInput")
    y_out = nc.dram_tensor("y", [SHARD, OUT_C], f32, kind="ExternalOutput")

    cc_in = [nc.dram_tensor(f"cc_in{l}", [SHARD, HID], f32) for l in range(n_layers)]
    cc_out = [nc.dram_tensor(f"cc_out{l}", [NTOT, HID], f32, addr_space="Shared")
              for l in range(n_layers)]

    with tile.TileContext(nc) as tc:
        with (
            tc.tile_pool(name="const", bufs=1) as constp,
            tc.tile_pool(name="h0p", bufs=1) as h0p,
            tc.tile_pool(name="idxp", bufs=1) as idxrp,
            tc.tile_pool(name="gp", bufs=6) as gp,
            tc.tile_pool(name="wk", bufs=10) as wk,
            tc.tile_pool(name="xp", bufs=2) as xp,
            tc.tile_pool(name="psum", bufs=2, space="PSUM") as psp,
        ):
            ident = constp.tile([128, 128], f32)
            make_identity(nc, ident[:])
            ones1 = constp.tile([1, 128], f32)
            nc.vector.memset(ones1[:], 1.0)
            l0T = constp.tile([IN_C, HID], f32)
            nc.sync.dma_start(out=l0T[:], in_=l0T_in[:])
            b0t = constp.tile([1, HID], f32)
            nc.sync.dma_start(out=b0t[:], in_=b0_in[:])
            l1T = constp.tile([HID, OUT_C], f32)
            nc.sync.dma_start(out=l1T[:], in_=l1T_in[:])
            b1t = constp.tile([1, OUT_C], f32)
            nc.sync.dma_start(out=b1t[:], in_=b1_in[:])
            wp = constp.tile([HID, N_LAYERS * HID], f32)
            nc.sync.dma_start(out=wp[:], in_=wp_in[:])
            idxs = idxrp.tile([128, 8 * tot], i16)
            nc.sync.dma_start(out=idxs[:], in_=idx_in[:])
            h0s = h0p.tile([128, NTILE * HID], f32)   # 0.1*h0 per tile
            ones84 = constp.tile([1, 128], f32)
            nc.vector.memset(ones84[:], 1.0)
            nc.vector.memset(ones84[:, 84:], 0.0)
            # ---------- prologue: h0 = relu(x @ lin0^T + b0) ----------
            for t in range(NTILE):
                xt = xp.tile([IN_C, 128], f32, tag="xt")
                nc.sync.dma_start(out=xt[:], in_=xT_in[:, t * 128:(t + 1) * 128])
                ph = psp.tile([128, HID], f32, tag="ph")
                nc.tensor.matmul(out=ph[:], lhsT=xt[:], rhs=l0T[:], start=True, stop=False)
                nc.tensor.matmul(out=ph[:], lhsT=(ones84 if t == NTILE - 1 else ones1)[:],
                                 rhs=b0t[:], start=False, stop=True)
                h0t = wk.tile([128, HID], f32, tag="h0t")
                nc.scalar.activation(out=h0t[:], in_=ph[:], func=mybir.ActivationFunctionType.Relu)
                nc.scalar.mul(out=h0s[:, t * HID:(t + 1) * HID], in_=h0t[:], mul=0.1)
                nc.sync.dma_start(out=cc_in[0][t * 128:(t + 1) * 128, :], in_=h0t[:])

            for l in range(n_layers):
                nc.gpsimd.collective_compute(
                    "AllGather", mybir.AluOpType.bypass,
                    ins=[cc_in[l][:]], outs=[cc_out[l][:]],
                    replica_groups=[list(range(NC_CORES))],
                )
                table = cc_out[l]
                ci = 0
                for t in range(NTILE):
                    slots = int(S_t[t])
                    if slots:
                        g = gp.tile([128, gmax, HID], f32, tag="g")
                    while ci < len(calls) and calls[ci][0] == t:
                        (_, w, coff, S) = calls[ci]
                        c0 = 8 * (int(goff[t]) + coff)
                        nc.gpsimd.dma_gather(
                            out_ap=g[:, coff:coff + S, :],
                            in_ap=table[w * WIN:, :] if w else table[:],
                            idxs_ap=idxs[:, c0:c0 + 8 * S],
                            num_idxs=128 * S,
                            num_idxs_reg=128 * S,
                            elem_size=HID,
                            queue_num=w,
                        )
                        ci += 1
                    # tree-reduce slots -> slot 0
                    S_cur = slots
                    while S_cur > 1:
                        h = S_cur // 2
                        nc.vector.tensor_add(
                            out=g[:, 0:h, :],
                            in0=g[:, 0:h, :],
                            in1=g[:, S_cur - h:S_cur, :],
                        )
                        S_cur -= h
                    z = wk.tile([128, HID], f32, tag="z")
                    if slots:
                        nc.scalar.mul(out=z[:], in_=g[:, 0, :], mul=1.0 - ALPHA)
                        nc.vector.tensor_add(out=z[:], in0=z[:],
                                             in1=h0s[:, t * HID:(t + 1) * HID])
                    else:
                        nc.vector.tensor_copy(out=z[:], in_=h0s[:, t * HID:(t + 1) * HID])
                    pzT = psp.tile([HID, 128], f32, tag="pzT")
                    nc.tensor.transpose(out=pzT[:], in_=z[:], identity=ident[:])
                    zT = wk.tile([HID, 128], f32, tag="zT")
                    nc.vector.tensor_copy(out=zT[:], in_=pzT[:])
                    ph2 = psp.tile([128, HID], f32, tag="ph2")
                    nc.tensor.matmul(out=ph2[:], lhsT=zT[:],
                                     rhs=wp[:, l * HID:(l + 1) * HID],
                                     start=True, stop=True)
                    ht = wk.tile([128, HID], f32, tag="ht")
                    nc.scalar.activation(out=ht[:], in_=ph2[:],
                                         func=mybir.ActivationFunctionType.Relu)
                    if l + 1 < n_layers:
                        nc.sync.dma_start(out=cc_in[l + 1][t * 128:(t + 1) * 128, :], in_=ht[:])
                    else:
                        phT = psp.tile([HID, 128], f32, tag="pzT")
                        nc.tensor.transpose(out=phT[:], in_=ht[:], identity=ident[:])
                        hT = wk.tile([HID, 128], f32, tag="zT")
                        nc.vector.tensor_copy(out=hT[:], in_=phT[:])
                        py = psp.tile([128, OUT_C], f32, tag="ph2")
                        nc.tensor.matmul(out=py[:], lhsT=hT[:], rhs=l1T[:], start=True, stop=False)
                        nc.tensor.matmul(out=py[:], lhsT=ones1[:], rhs=b1t[:], start=False, stop=True)
                        yt = wk.tile([128, OUT_C], f32, tag="ht")
                        nc.vector.tensor_copy(out=yt[:], in_=py[:])
                        nc.sync.dma_start(out=y_out[t * 128:(t + 1) * 128, :], in_=yt[:])
    nc.finalize()
    return nc


LAST_RESULT = None  # BassKernelResults of the most recent run (for test harness)


def kernel(x, edge_index, lin0_w, lin0_b, lin1_w, lin1_b, conv_w, _layers=N_LAYERS):
    from concourse.bass_utils import run_bass_kernel_spmd

    x = np.asarray(x, dtype=np.float32)
    meta = _host_prep(edge_index)
    newid = meta["newid"]

    nc = _build_nc(meta, _layers)

    betas = np.log(THETA / (np.arange(N_LAYERS) + 1) + 1.0).astype(np.float32)
    wp = np.concatenate(
        [((1 - b) * np.eye(HID, dtype=np.float32) + b * np.asarray(conv_w[l], np.float32))
         for l, b in enumerate(betas)], axis=1)
    common = {
        "lin0T": np.ascontiguousarray(np.asarray(lin0_w, np.float32).T),
        "b0": np.asarray(lin0_b, np.float32).reshape(1, HID),
        "lin1T": np.ascontiguousarray(np.asarray(lin1_w, np.float32).T),
        "b1": np.asarray(lin1_b, np.float32).reshape(1, OUT_C),
        "wp": np.ascontiguousarray(wp),
    }
    in_maps = []
    for c in range(NC_CORES):
        xT = np.zeros((IN_C, SHARD), dtype=np.float32)
        ids = np.where(newid // SHARD == c)[0]
        xT[:, newid[ids] % SHARD] = x[ids].T
        in_maps.append({"xT": np.ascontiguousarray(xT),
                        "idx": meta["idx_res"][c], **common})

    res = run_bass_kernel_spmd(nc, in_maps, list(range(NC_CORES)))
    global LAST_RESULT
    LAST_RESULT = res
    y_full = np.concatenate([res.results[c]["y"] for c in range(NC_CORES)], axis=0)
    return np.ascontiguousarray(y_full[newid])



# revision 24
# speedup vs baseline: 1.0810x; 1.0810x over previous
"""GCN2-encoder Trainium kernel: 8-core SPMD bass/Tile implementation (v3).

Key structure:
 - dst-core-pair assignment via balanced 4-coloring of sources so each dst's
   in-neighbors split evenly across the 4 gather windows (padding 2.16x->1.4x).
 - gather indices are SBUF-resident, loaded once and reused by all 16 layers.
 - per-layer Shared-output AllGather replicates the h table to all cores.
 - per-tile: dma_gather per (tile,window) chunk -> DVE tree-add -> z =
   0.9*agg + 0.1*h0 -> PE transpose -> matmul with folded W'_l -> relu.
 - v3 perf: SWDGE descriptor generation is the bottleneck (Q7 core-pair per
   queue); num_swdge_queues=4 + queue_num=w spreads desc-gen over all four
   Q7 core-pairs (one per window), and deeper gather/work pools keep the
   GPSIMD queue fed.  40.0ms -> 14.6ms.  NOTE: dma_gather calls are capped
   at 1024 idxs (64+1 descs/engine-ring); bigger calls overflow the per-queue
   SWDGE descriptor ring carveout and hang the device.
"""
import numpy as np

N_NODES = 100000
N_EDGES = 1600000
IN_C, HID, OUT_C = 128, 64, 64
N_LAYERS = 16
ALPHA, THETA = 0.1, 0.5

NC_CORES = 8
SHARD = 12544            # 98 tiles of 128
NTILE = SHARD // 128     # 98
NTOT = NC_CORES * SHARD  # 100352
WIN = 25088              # window = one core-pair's rows
NWIN = 4
ZERO_REL = 12540         # window-relative row that is always zero (pad of even core)
CHUNK = 8                # max slots (x128 idxs) per dma_gather call


def _host_prep(edge_index):
    src = np.asarray(edge_index[0], dtype=np.int64)
    dst = np.asarray(edge_index[1], dtype=np.int64)
    deg = np.bincount(dst, minlength=N_NODES)
    rng = np.random.default_rng(0)

    # ---- balanced 4-coloring of sources (color = dst core-pair = window) ----
    # Sequential greedy: recolor each node to minimize squared excess of its
    # out-neighbors' per-window in-degree above ceil(deg/4).
    out_order = np.argsort(src, kind="stable")
    odst = dst[out_order]
    starts = np.searchsorted(src[out_order], np.arange(N_NODES))
    ends = np.r_[starts[1:], N_EDGES]
    color = rng.integers(0, NWIN, N_NODES).astype(np.int64)
    cnt = np.zeros((N_NODES, NWIN), dtype=np.int64)
    np.add.at(cnt, (dst, color[src]), 1)
    q = -(-deg // NWIN)  # per-dst per-window quota
    for it in range(16):
        changed = 0
        for u in range(N_NODES):
            s, e = starts[u], ends[u]
            if s == e:
                continue
            ds = odst[s:e]
            cu = color[u]
            base = cnt[ds].copy()
            base[:, cu] -= 1
            ov = base + 1 - q[ds][:, None]
            costs = (np.minimum(np.maximum(ov, 0), 8) ** 2
                     - np.minimum(np.maximum(ov - 1, 0), 8) ** 2).sum(axis=0)
            c2 = int(np.argmin(costs))
            if c2 != cu and costs[c2] < costs[cu]:
                np.add.at(cnt, (ds, cu), -1)
                np.add.at(cnt, (ds, c2), 1)
                color[u] = c2
                changed += 1
        if changed < 100:
            break
    sizes = np.bincount(color, minlength=NWIN)
    # hard-rebalance: each color must hold EXACTLY 25000 real nodes so every
    # core gets exactly 12500 (the shared program masks pads at column 84 of
    # the last tile).
    for _ in range(8):
        if (sizes == 25000).all():
            break
        w = int(np.argmax(sizes))
        tgt = int(np.argmin(sizes))
        nodes_w = np.where(color == w)[0]
        k = min(int(sizes[w] - 25000), int(25000 - sizes[tgt]))
        mv = nodes_w[np.argsort(deg[nodes_w])[:k]]
        inb2 = np.zeros(N_NODES, dtype=bool)
        inb2[mv] = True
        m2 = inb2[src]
        es2, ed2 = src[m2], dst[m2]
        np.add.at(cnt, (ed2, color[es2]), -1)
        sizes[w] -= k
        color[mv] = tgt
        sizes[tgt] += k
        np.add.at(cnt, (ed2, color[es2]), 1)
    assert (np.bincount(color, minlength=NWIN) == 25000).all()
    # recompute exactly (duplicate edges make incremental updates drift)
    cnt = np.zeros((N_NODES, NWIN), dtype=np.int64)
    np.add.at(cnt, (dst, color[src]), 1)

    # ---- pair -> cores: alternate within each color by profile sort key ----
    # Sort key clusters nodes with similar degree and deviation signature so
    # per-tile per-window maxima stay close to the mean.
    dev = cnt - q[:, None]
    heavy = dev.argmax(axis=1)
    srt = np.sort(dev, axis=1)[:, ::-1]
    skey = ((((deg * 64 + (srt[:, 0] + 16)) * 4 + heavy) * 64
             + (srt[:, 1] + 16)) * 64 + (srt[:, 2] + 16))
    core_of = np.empty(N_NODES, dtype=np.int64)
    newid = np.empty(N_NODES, dtype=np.int64)
    for w in range(NWIN):
        nodes_w = np.where(color == w)[0]
        o = nodes_w[np.argsort(-skey[nodes_w], kind="stable")]
        core_of[o] = 2 * w + (np.arange(len(o)) % 2)
    for c in range(NC_CORES):
        nodes_c = np.where(core_of == c)[0]
        o = nodes_c[np.argsort(-skey[nodes_c], kind="stable")]
        newid[o] = c * SHARD + np.arange(len(nodes_c))
        assert len(nodes_c) <= SHARD - 4, f"core {c}: {len(nodes_c)}"

    src_new = newid[src]
    dst_new = newid[dst]

    # ---- per (tile, window) padded degree, unified across cores ----
    slot_node = np.full(NTOT, -1, dtype=np.int64)
    slot_node[newid] = np.arange(N_NODES)
    D = np.zeros((NTILE, NWIN), dtype=np.int64)
    for t in range(NTILE):
        ids = slot_node[(np.arange(NC_CORES)[:, None] * SHARD + t * 128
                         + np.arange(128)[None, :]).reshape(-1)]
        ids = ids[ids >= 0]
        if len(ids):
            D[t] = cnt[ids].max(axis=0)
    S_t = D.sum(axis=1)                     # slots per tile
    gmax = int(S_t.max())
    tot_slots = int(S_t.sum())

    # ---- call plan (shared): (t, w, tile-local col off, S) ----
    calls = []
    for t in range(NTILE):
        off = 0
        for w in range(NWIN):
            d = int(D[t, w])
            s0 = 0
            while s0 < d:
                S = min(CHUNK, d - s0)
                # overflow chunks hop to the opposite queue (avoid HOL)
                calls.append((t, w, off + s0, S, (w + 2 * (s0 > 0)) % NWIN))
                s0 += S
            off += d
    # global slot offset of each tile in the resident idx stream
    goff = np.zeros(NTILE + 1, dtype=np.int64)
    goff[1:] = np.cumsum(S_t)

    # ---- per-core idx streams (resident layout [128, 8*tot_slots] i16) ----
    # edge -> (core, tile, p, w); slot within tile = woff[t,w] + rank
    woff = np.zeros((NTILE, NWIN), dtype=np.int64)
    woff[:, 1:] = np.cumsum(D, axis=1)[:, :-1]
    core_e = dst_new // SHARD
    tile_e = (dst_new % SHARD) // 128
    p_e = dst_new % 128
    w_e = src_new // WIN
    rel_e = (src_new - w_e * WIN).astype(np.int16)
    # rank within (core, tile, w, p)
    ek = ((core_e * NTILE + tile_e) * NWIN + w_e) * 128 + p_e
    eo = np.argsort(ek, kind="stable")
    ek_s = ek[eo]
    grp = np.r_[0, np.flatnonzero(np.diff(ek_s)) + 1]
    rank = np.arange(N_EDGES) - np.repeat(grp, np.diff(np.r_[grp, N_EDGES]))

    idx_flat = np.full((NC_CORES, tot_slots * 128), ZERO_REL, dtype=np.int16)
    # ZERO_REL must differ per window? zero row rel offset is the same (12540)
    # position of edge within core stream: slot = goff[t] + woff[t,w] + rank
    slot_e = goff[tile_e[eo]] + woff[tile_e[eo], w_e[eo]] + rank
    pos_e = slot_e * 128 + p_e[eo]
    idx_flat[core_e[eo], pos_e] = rel_e[eo]

    # wrap into [128, 8*tot_slots]: position j -> (16-wrap, replicated x8)
    # within each slot s: 128 positions j=s*128+p -> columns [8s, 8s+8) rows 16
    idx_res = np.empty((NC_CORES, 128, 8 * tot_slots), dtype=np.int16)
    a = idx_flat.reshape(NC_CORES, tot_slots, 8, 16)  # [c, s, j16, 16]
    aw = a.transpose(0, 3, 1, 2).reshape(NC_CORES, 16, tot_slots * 8)
    idx_res[:] = np.tile(aw, (1, 8, 1))

    return dict(newid=newid, calls=calls, D=D, S_t=S_t, gmax=gmax,
                tot_slots=tot_slots, goff=goff, idx_res=idx_res)


def _build_nc(meta, n_layers):
    import concourse.bacc as bacc
    import concourse.mybir as mybir
    import concourse.tile as tile
    from concourse.masks import make_identity

    calls = meta["calls"]
    S_t = meta["S_t"]
    gmax = meta["gmax"]
    tot = meta["tot_slots"]
    goff = meta["goff"]
    f32 = mybir.dt.float32
    i16 = mybir.dt.int16

    nc = bacc.Bacc(None, num_swdge_queues=4)
    xT_in = nc.dram_tensor("xT", [IN_C, SHARD], f32, kind="ExternalInput")
    idx_in = nc.dram_tensor("idx", [128, 8 * tot], i16, kind="ExternalInput")
    l0T_in = nc.dram_tensor("lin0T", [IN_C, HID], f32, kind="ExternalInput")
    b0_in = nc.dram_tensor("b0", [1, HID], f32, kind="ExternalInput")
    l1T_in = nc.dram_tensor("lin1T", [HID, OUT_C], f32, kind="ExternalInput")
    b1_in = nc.dram_tensor("b1", [1, OUT_C], f32, kind="ExternalInput")
    wp_in = nc.dram_tensor("wp", [HID, N_LAYERS * HID], f32, kind="ExternalInput")
    y_out = nc.dram_tensor("y", [SHARD, OUT_C], f32, kind="ExternalOutput")

    cc_in = [nc.dram_tensor(f"cc_in{l}", [SHARD, HID], f32) for l in range(n_layers)]
    cc_out = [nc.dram_tensor(f"cc_out{l}", [NTOT, HID], f32, addr_space="Shared")
              for l in range(n_layers)]

    with tile.TileContext(nc) as tc:
        with (
            tc.tile_pool(name="const", bufs=1) as constp,
            tc.tile_pool(name="h0p", bufs=1) as h0p,
            tc.tile_pool(name="idxp", bufs=1) as idxrp,
            tc.tile_pool(name="gp", bufs=6) as gp,
            tc.tile_pool(name="wk", bufs=10) as wk,
            tc.tile_pool(name="xp", bufs=2) as xp,
            tc.tile_pool(name="psum", bufs=2, space="PSUM") as psp,
        ):
            ident = constp.tile([128, 128], f32)
            make_identity(nc, ident[:])
            ones1 = constp.tile([1, 128], f32)
            nc.vector.memset(ones1[:], 1.0)
            l0T = constp.tile([IN_C, HID], f32)
            nc.sync.dma_start(out=l0T[:], in_=l0T_in[:])
            b0t = constp.tile([1, HID], f32)
            nc.sync.dma_start(out=b0t[:], in_=b0_in[:])
            l1T = constp.tile([HID, OUT_C], f32)
            nc.sync.dma_start(out=l1T[:], in_=l1T_in[:])
            b1t = constp.tile([1, OUT_C], f32)
            nc.sync.dma_start(out=b1t[:], in_=b1_in[:])
            wp = constp.tile([HID, N_LAYERS * HID], f32)
            nc.sync.dma_start(out=wp[:], in_=wp_in[:])
            idxs = idxrp.tile([128, 8 * tot], i16)
            nc.sync.dma_start(out=idxs[:], in_=idx_in[:])
            h0s = h0p.tile([128, NTILE * HID], f32)   # 0.1*h0 per tile
            ones84 = constp.tile([1, 128], f32)
            nc.vector.memset(ones84[:], 1.0)
            nc.vector.memset(ones84[:, 84:], 0.0)
            # ---------- prologue: h0 = relu(x @ lin0^T + b0) ----------
            for t in range(NTILE):
                xt = xp.tile([IN_C, 128], f32, tag="xt")
                nc.sync.dma_start(out=xt[:], in_=xT_in[:, t * 128:(t + 1) * 128])
                ph = psp.tile([128, HID], f32, tag="ph")
                nc.tensor.matmul(out=ph[:], lhsT=xt[:], rhs=l0T[:], start=True, stop=False)
                nc.tensor.matmul(out=ph[:], lhsT=(ones84 if t == NTILE - 1 else ones1)[:],
                                 rhs=b0t[:], start=False, stop=True)
                h0t = wk.tile([128, HID], f32, tag="h0t")
                nc.scalar.activation(out=h0t[:], in_=ph[:], func=mybir.ActivationFunctionType.Relu)
                nc.scalar.mul(out=h0s[:, t * HID:(t + 1) * HID], in_=h0t[:], mul=0.1)
                nc.sync.dma_start(out=cc_in[0][t * 128:(t + 1) * 128, :], in_=h0t[:])

            for l in range(n_layers):
                nc.gpsimd.collective_compute(
                    "AllGather", mybir.AluOpType.bypass,
                    ins=[cc_in[l][:]], outs=[cc_out[l][:]],
                    replica_groups=[list(range(NC_CORES))],
                )
                table = cc_out[l]
                ci = 0
                for t in range(NTILE):
                    slots = int(S_t[t])
                    if slots:
                        g = gp.tile([128, gmax, HID], f32, tag="g")
                    while ci < len(calls) and calls[ci][0] == t:
                        (_, w, coff, S, qn) = calls[ci]
                        c0 = 8 * (int(goff[t]) + coff)
                        nc.gpsimd.dma_gather(
                            out_ap=g[:, coff:coff + S, :],
                            in_ap=table[w * WIN:, :] if w else table[:],
                            idxs_ap=idxs[:, c0:c0 + 8 * S],
                            num_idxs=128 * S,
                            num_idxs_reg=128 * S,
                            elem_size=HID,
                            queue_num=qn,
                        )
                        ci += 1
                    # tree-reduce slots -> slot 0
                    S_cur = slots
                    while S_cur > 1:
                        h = S_cur // 2
                        nc.vector.tensor_add(
                            out=g[:, 0:h, :],
                            in0=g[:, 0:h, :],
                            in1=g[:, S_cur - h:S_cur, :],
                        )
                        S_cur -= h
                    z = wk.tile([128, HID], f32, tag="z")
                    if slots:
                        nc.scalar.mul(out=z[:], in_=g[:, 0, :], mul=1.0 - ALPHA)
                        nc.vector.tensor_add(out=z[:], in0=z[:],
                                             in1=h0s[:, t * HID:(t + 1) * HID])
                    else:
                        nc.vector.tensor_copy(out=z[:], in_=h0s[:, t * HID:(t + 1) * HID])
                    pzT = psp.tile([HID, 128], f32, tag="pzT")
                    nc.tensor.transpose(out=pzT[:], in_=z[:], identity=ident[:])
                    zT = wk.tile([HID, 128], f32, tag="zT")
                    nc.vector.tensor_copy(out=zT[:], in_=pzT[:])
                    ph2 = psp.tile([128, HID], f32, tag="ph2")
                    nc.tensor.matmul(out=ph2[:], lhsT=zT[:],
                                     rhs=wp[:, l * HID:(l + 1) * HID],
                                     start=True, stop=True)
                    ht = wk.tile([128, HID], f32, tag="ht")
                    nc.scalar.activation(out=ht[:], in_=ph2[:],
                                         func=mybir.ActivationFunctionType.Relu)
                    if l + 1 < n_layers:
                        nc.sync.dma_start(out=cc_in[l + 1][t * 128:(t + 1) * 128, :], in_=ht[:])
                    else:
                        phT = psp.tile([HID, 128], f32, tag="pzT")
                        nc.tensor.transpose(out=phT[:], in_=ht[:], identity=ident[:])
                        hT = wk.tile([HID, 128], f32, tag="zT")
                        nc.vector.tensor_copy(out=hT[:], in_=phT[:])
                        py = psp.tile([128, OUT_C], f32, tag="ph2")
                        nc.tensor.matmul(out=py[:], lhsT=hT[:], rhs=l1T[:], start=True, stop=False)
                        nc.tensor.matmul(out=py[:], lhsT=ones1[:], rhs=b1t[:], start=False, stop=True)
                        yt = wk.tile([128, OUT_C], f32, tag="ht")
                        nc.vector.tensor_copy(out=yt[:], in_=py[:])
                        nc.sync.dma_start(out=y_out[t * 128:(t + 1) * 128, :], in_=yt[:])
    nc.finalize()
    return nc


LAST_RESULT = None  # BassKernelResults of the most recent run (for test harness)


def kernel(x, edge_index, lin0_w, lin0_b, lin1_w, lin1_b, conv_w, _layers=N_LAYERS):
    from concourse.bass_utils import run_bass_kernel_spmd

    x = np.asarray(x, dtype=np.float32)
    meta = _host_prep(edge_index)
    newid = meta["newid"]

    nc = _build_nc(meta, _layers)

    betas = np.log(THETA / (np.arange(N_LAYERS) + 1) + 1.0).astype(np.float32)
    wp = np.concatenate(
        [((1 - b) * np.eye(HID, dtype=np.float32) + b * np.asarray(conv_w[l], np.float32))
         for l, b in enumerate(betas)], axis=1)
    common = {
        "lin0T": np.ascontiguousarray(np.asarray(lin0_w, np.float32).T),
        "b0": np.asarray(lin0_b, np.float32).reshape(1, HID),
        "lin1T": np.ascontiguousarray(np.asarray(lin1_w, np.float32).T),
        "b1": np.asarray(lin1_b, np.float32).reshape(1, OUT_C),
        "wp": np.ascontiguousarray(wp),
    }
    in_maps = []
    for c in range(NC_CORES):
        xT = np.zeros((IN_C, SHARD), dtype=np.float32)
        ids = np.where(newid // SHARD == c)[0]
        xT[:, newid[ids] % SHARD] = x[ids].T
        in_maps.append({"xT": np.ascontiguousarray(xT),
                        "idx": meta["idx_res"][c], **common})

    res = run_bass_kernel_spmd(nc, in_maps, list(range(NC_CORES)))
    global LAST_RESULT
    LAST_RESULT = res
    y_full = np.concatenate([res.results[c]["y"] for c in range(NC_CORES)], axis=0)
    return np.ascontiguousarray(y_full[newid])

